# revision 24
# baseline (speedup 1.0000x reference)
"""DigitCaps (capsule routing) Trainium2 Bass kernel.

u [512, 1152, 8] f32, W [1, 1152, 10, 16, 8] f32 -> v [512, 10, 16] f32
(3 dynamic-routing iterations, softmax over 10 classes).

Pure data-parallel: batch 64 per core x 8 cores; everything on-chip;
u_hat (377MB) is never materialized.

Agreement phase per routing iteration (k-major pipeline):
  T^T[(k4,i32), (ch,b)] = sum_d W[i,c,d,k] v[b,c,d]   PE (lhsT = W slice
                                    [32=(ch,d), (k4,i32)], rhs = block-diag
                                    v^T), f32 PSUM, evac to bf16 (ACT/Pool)
  P^T = T^T * u^T                                     DVE (bf16, 2x mode)
  agr[r, (g,ch,b)] = sum_k P^T                        PE ones-matmuls: the
                                    delta-block lhsT contracts k' across
                                    partitions; out partition offset 32*jq
                                    reassembles exact i-major rows; the two
                                    k-halves accumulate in PSUM
  cexp *= exp(agr)    (cexp is the routing state; logits never materialized)
S phase (i-major, as before):
  xc = u*rec*cexp_c   DVE/Pool per class
  s[b,c,:] = sum_(ik) W xc                            PE (72 matmuls/class)
  v = squash(s)
"""

import numpy as np

N_CORES = 8
B_PER = 64
I_CAPS = 1152
K_DIM = 8
C_CLS = 10
D_DIM = 16
NG = I_CAPS // 128  # 9
NJ = I_CAPS // 32   # 36 i-chunks of 32
EPS = 1e-8

# tuning knobs: engine split
XC_POOL_CLASSES = 2      # of 10 xc multiplies -> Pool (rest DVE)
DIRECT_GROUPS = 3        # of 12 (h,j0)-groups per pass: DVE reads PSUM direct
import os as _os
INTERLEAVE_ONES = _os.environ.get("IL_ONES", "0") == "1"

_CACHE = {}


def _build():
    import concourse.bass as bass
    import concourse.mybir as mybir
    from concourse import tile, bacc

    f32 = mybir.dt.float32
    bf16 = mybir.dt.bfloat16
    AF = mybir.ActivationFunctionType
    OP = mybir.AluOpType

    nc = bacc.Bacc()
    uTk_in = nc.dram_tensor(
        "uTk_h", [128, K_DIM, NG, B_PER], bf16, kind="ExternalInput"
    )
    uT2_in = nc.dram_tensor(
        "uT2_h", [128, 2, NJ, B_PER], bf16, kind="ExternalInput"
    )
    wsk_in = nc.dram_tensor(
        "wsk_h", [128, K_DIM, NG, C_CLS, D_DIM], bf16, kind="ExternalInput"
    )
    wt_in = nc.dram_tensor("wt_h", [128, 2, NJ, 128], bf16, kind="ExternalInput")
    wtb_in = nc.dram_tensor("wtb_h", [128, 2, NJ, 128], bf16, kind="ExternalInput")
    ones_in = nc.dram_tensor("ones32", [128, 32], bf16, kind="ExternalInput")
    eye128 = nc.dram_tensor("eye128", [128, 128], f32, kind="ExternalInput")
    v_out = nc.dram_tensor("v", [B_PER, C_CLS, D_DIM], f32, kind="ExternalOutput")

    with tile.TileContext(nc) as tc:
        perm = tc.alloc_tile_pool(name="perm", bufs=1)
        Wsk = perm.tile([128, K_DIM, NG, C_CLS, D_DIM], bf16)  # [r,(k,g,c,d)]
        WT = perm.tile([128, 2, NJ, 128], bf16)   # [16c+d, h, jj, (k',i'')] c0-7
        WTB = perm.tile([128, 2, NJ, 128], bf16)  # same, rows 16(c-2)+d; 96:128 used
        uTk = perm.tile([128, K_DIM, NG, B_PER], bf16)      # u[b, 128g+r, k]
        uT2 = perm.tile([128, 2, NJ, B_PER], bf16)   # u[b, 32j+i'', 4h+k']
        ones32 = perm.tile([128, 32], bf16)          # delta(i'' == m)
        cE = perm.tile([128, 5, NG, 2, B_PER], bf16, name="cEt")  # exp-state
        recT = perm.tile([128, NG, B_PER], bf16, name="recTt")    # 1/den
        vT = perm.tile([128, 128], bf16)             # block-diag v^T classes 0-7
        vT4 = perm.tile([128, 128], bf16)            # rows 96:128: classes 8,9
        v_sb = perm.tile([64, C_CLS, D_DIM], f32, name="vsbt")
        s_sb = perm.tile([64, C_CLS, D_DIM], f32, name="ssbt")
        eye_sb = perm.tile([128, 128], f32)
        in2 = perm.tile([128, 128], f32)
        in2b = perm.tile([128, 128], f32)
        sq = perm.tile([64, C_CLS, D_DIM], f32)
        n2 = perm.tile([64, C_CLS], f32)
        t1 = perm.tile([64, C_CLS], f32)
        r1 = perm.tile([64, C_CLS], f32)
        f1 = perm.tile([64, C_CLS], f32)
        nrm = perm.tile([64, C_CLS], f32)
        nrm2 = perm.tile([64, C_CLS], f32)
        r2 = perm.tile([64, C_CLS], f32)
        fac = perm.tile([64, C_CLS], f32)
        Tsb = perm.tile([128, 2, NJ, 128], bf16, name="Tsbt")  # evac'd T^T
        PT = perm.tile([128, 2, NJ, 128], bf16, name="PTt")    # T^T * u^T

        psS = tc.alloc_tile_pool(name="psS", bufs=1, space="PSUM")
        psT = tc.alloc_tile_pool(name="psT", bufs=2, space="PSUM")
        psA = tc.alloc_tile_pool(name="psA", bufs=1, space="PSUM")

        # ---------------- setup: inputs arrive pre-arranged ----
        nc.sync.dma_start(uTk[:, 0:4], uTk_in[:, 0:4])
        nc.scalar.dma_start(Wsk[:, 0:4], wsk_in[:, 0:4])
        nc.sync.dma_start(uTk[:, 4:8], uTk_in[:, 4:8])
        nc.scalar.dma_start(Wsk[:, 4:8], wsk_in[:, 4:8])
        nc.sync.dma_start(eye_sb[:], eye128[:])
        nc.sync.dma_start(ones32[:], ones_in[:])
        nc.scalar.dma_start(WT[:, 0], wt_in[:, 0])
        nc.sync.dma_start(uT2[:], uT2_in[:])
        nc.scalar.dma_start(WTB[:, 0], wtb_in[:, 0])
        nc.sync.dma_start(WT[:, 1], wt_in[:, 1])
        nc.scalar.dma_start(WTB[:, 1], wtb_in[:, 1])

        nc.vector.memset(in2[:], 0.0)
        nc.vector.memset(in2b[:], 0.0)

        smp = tc.alloc_tile_pool(name="smp", bufs=3)

        def s_phase_s0():
            ps = psS.tile([64, C_CLS * D_DIM], f32, tag="ps_s")
            n = 0
            for k in range(K_DIM):
                for g in range(NG):
                    nc.tensor.matmul(
                        ps[:],
                        uTk[:, k, g, :],
                        Wsk[:, k, g, :, :].rearrange("r c d -> r (c d)"),
                        start=(n == 0),
                        stop=(n == K_DIM * NG - 1),
                    )
                    n += 1
            nc.scalar.activation(
                s_sb[:].rearrange("b c d -> b (c d)"), ps[:], AF.Copy, scale=0.1
            )

        def squash(final):
            # fac = sqrt(n2)/(1+n2)  (the +eps on the norm is negligible)
            nc.scalar.square(sq[:], s_sb[:])
            nc.vector.reduce_sum(n2[:], sq[:], axis=mybir.AxisListType.X)
            nc.scalar.sqrt(nrm[:], n2[:])
            nc.vector.tensor_scalar_add(t1[:], n2[:], 1.0)
            nc.vector.reciprocal(r1[:], t1[:])
            nc.vector.tensor_mul(fac[:], nrm[:], r1[:])
            if final:
                nc.vector.tensor_tensor(
                    v_sb[:],
                    s_sb[:],
                    fac[:].rearrange("b c -> b c ()").to_broadcast(
                        (64, C_CLS, D_DIM)
                    ),
                    OP.mult,
                )
            else:
                # write v directly into the transpose sources (skip v_sb)
                i2v = in2[:].rearrange("q (c d) -> q c d", d=D_DIM)
                for q0, cs in ((0, 0), (64, 1)):
                    nc.vector.tensor_tensor(
                        i2v[q0 : q0 + 64, cs::2, :],
                        s_sb[:, cs:8:2, :],
                        fac[:, cs:8:2].rearrange("b c -> b c ()").to_broadcast(
                            (64, 4, D_DIM)
                        ),
                        OP.mult,
                    )
                nc.vector.tensor_tensor(
                    in2b[0:64, 96:112].rearrange("b (c d) -> b c d", c=1),
                    s_sb[:, 8:9, :],
                    fac[:, 8:9].rearrange("b c -> b c ()").to_broadcast(
                        (64, 1, D_DIM)
                    ),
                    OP.mult,
                )
                nc.vector.tensor_tensor(
                    in2b[64:128, 112:128].rearrange("b (c d) -> b c d", c=1),
                    s_sb[:, 9:10, :],
                    fac[:, 9:10].rearrange("b c -> b c ()").to_broadcast(
                        (64, 1, D_DIM)
                    ),
                    OP.mult,
                )

        def build_vT():
            # in2/in2b already hold v (written by squash)
            pv = psT.tile([128, 768], f32, tag="pt")
            nc.tensor.transpose(pv[:, 0:128], in2[:], eye_sb[:])
            nc.vector.tensor_copy(vT[:], pv[:, 0:128])
            pv4 = psT.tile([128, 768], f32, tag="pt")
            nc.tensor.transpose(pv4[:, 0:128], in2b[:], eye_sb[:])
            nc.scalar.copy(vT4[:], pv4[:, 0:128])

        def emit_ones(agrP, ja, jb):
            for j in range(ja, jb):
                g, jq = j // 4, j % 4
                for hh in range(2):
                    nc.tensor.matmul(
                        agrP[32 * jq : 32 * jq + 32, g, :, :],
                        ones32[:],
                        PT[:, hh, j, :],
                        start=(hh == 0),
                        stop=(hh == 1),
                        skip_group_check=True,
                        tile_position=(0, 32 * jq),
                    )

        def agr_phase(it, bts):
            """One routing-agreement sweep: updates cexp (cE) and bts."""
            for p in range(5):
                vrhs = vT[32 * p : 32 * (p + 1), :] if p < 4 else vT4[96:128, :]
                row0 = 32 * p if p < 4 else 96
                agrP = psA.tile([128, NG, 2, B_PER], f32, tag="agr")
                # T^T matmuls + evac + mult, in groups of 6 j-chunks
                gidx = 0
                for h in range(2):
                    for j0 in range(0, NJ, 6):
                        tp = psT.tile([128, 768], f32, tag="pt")
                        for s in range(6):
                            jj = j0 + s
                            if p < 4:
                                lhs = WT[row0 : row0 + 32, h, jj, :]
                            else:
                                lhs = WTB[96:128, h, jj, :]
                            nc.tensor.matmul(
                                tp[:, 128 * s : 128 * (s + 1)],
                                lhs,
                                vrhs,
                                start=True,
                                stop=True,
                                tile_position=(row0, 0),
                            )
                        ub = uT2[:, h, j0 : j0 + 6, :].rearrange(
                            "r j b -> r j () b"
                        ).to_broadcast((128, 6, 2, B_PER))
                        pt_dst = PT[:, h, j0 : j0 + 6, :].rearrange(
                            "r j (c b) -> r j c b", c=2
                        )
                        if gidx % 4 == 3 and gidx // 4 < DIRECT_GROUPS:
                            # P^T = T^T(psum, f32) * u^T directly on DVE
                            nc.vector.tensor_tensor(
                                pt_dst,
                                tp[:].rearrange("r (j c b) -> r j c b", j=6, c=2),
                                ub,
                                OP.mult,
                            )
                        else:
                            # evac PSUM f32 -> SBUF bf16, then multiply
                            dst = Tsb[:, h, j0 : j0 + 6, :].rearrange(
                                "r j q -> r (j q)"
                            )
                            nc.scalar.copy(dst, tp[:])
                            nc.vector.tensor_tensor(
                                pt_dst,
                                Tsb[:, h, j0 : j0 + 6, :].rearrange(
                                    "r j (c b) -> r j c b", c=2
                                ),
                                ub,
                                OP.mult,
                            )
                        gidx += 1
                        if INTERLEAVE_ONES and h == 1 and j0 >= 6:
                            # k-sum for the PREVIOUS group (one-group lag so
                            # PE never waits on the DVE mult); i-major rows
                            # via out partition offset
                            emit_ones(agrP, j0 - 6, j0)
                if INTERLEAVE_ONES:
                    emit_ones(agrP, NJ - 6, NJ)
                else:
                    emit_ones(agrP, 0, NJ)
                # cexp update
                if it == 0:
                    nc.scalar.activation(
                        cE[:, p].rearrange("r g c b -> r (g c b)"),
                        agrP[:].rearrange("r g c b -> r (g c b)"),
                        AF.Exp,
                    )
                elif True:
                    Et = smp.tile([128, NG, 2, B_PER], bf16, tag="et", bufs=2)
                    nc.scalar.activation(
                        Et[:].rearrange("r g c b -> r (g c b)"),
                        agrP[:].rearrange("r g c b -> r (g c b)"),
                        AF.Exp,
                    )
                    nc.vector.tensor_tensor(
                        cE[:, p].rearrange("r g c b -> r (g c b)"),
                        cE[:, p].rearrange("r g c b -> r (g c b)"),
                        Et[:].rearrange("r g c b -> r (g c b)"),
                        OP.mult,
                    )
                # progressive softmax-denominator folds (overlap next pass)
                nc.vector.tensor_tensor(
                    bts[p][:], cE[:, p, :, 0, :], cE[:, p, :, 1, :], OP.add
                )
                if p == 1:
                    nc.vector.tensor_tensor(bts[0][:], bts[0][:], bts[1][:], OP.add)
                elif p == 3:
                    nc.vector.tensor_tensor(bts[2][:], bts[2][:], bts[3][:], OP.add)
                    nc.vector.tensor_tensor(bts[0][:], bts[0][:], bts[2][:], OP.add)

        def softmax_phase(bts):
            den = smp.tile([128, NG, B_PER], bf16, tag="smd", bufs=1)
            nc.vector.tensor_tensor(den[:], bts[0][:], bts[4][:], OP.add)
            with nc.allow_low_precision(reason="softmax reciprocal to bf16 ok"):
                nc.vector.reciprocal(
                    recT[:].rearrange("r g b -> r (g b)"),
                    den[:].rearrange("r g b -> r (g b)"),
                )

        itp = tc.alloc_tile_pool(name="itp", bufs=2)

        def s_phase_routed():
            # Pool-assigned classes: slow (2ns/elem) but fully overlapped --
            # their xc is issued first, their s-matmuls last.
            pool_cs = list(range(C_CLS - XC_POOL_CLASSES, C_CLS))
            dve_cs = list(range(C_CLS - XC_POOL_CLASSES))
            xcs = {}

            def issue_xc(c, eng, uTs):
                p, ch = c // 2, c % 2
                tag = f"xp{c}" if c >= C_CLS - XC_POOL_CLASSES else f"xc{c % 2}"
                xc = itp.tile([128, K_DIM, NG, B_PER], bf16, tag=tag, bufs=1)
                eng.tensor_tensor(
                    xc[:],
                    uTs[:],
                    cE[:, p, :, ch, :].rearrange("r g b -> r () g b").to_broadcast(
                        (128, K_DIM, NG, B_PER)
                    ),
                    OP.mult,
                )
                xcs[c] = xc

            def issue_s(c):
                xc = xcs[c]
                ps = psS.tile([64, C_CLS * D_DIM], f32, tag="ps_s")
                n = 0
                for k in range(K_DIM):
                    for g in range(NG):
                        nc.tensor.matmul(
                            ps[:, 16 * c : 16 * (c + 1)],
                            xc[:, k, g, :],
                            Wsk[:, k, g, c, :],
                            start=(n == 0),
                            stop=(n == K_DIM * NG - 1),
                        )
                        n += 1
                nc.scalar.copy(s_sb[:, c, :], ps[:, 16 * c : 16 * (c + 1)])

            uTs = itp.tile([128, K_DIM, NG, B_PER], bf16, tag="uts", bufs=1)
            nc.vector.tensor_tensor(
                uTs[:],
                uTk[:],
                recT[:].rearrange("r g b -> r () g b").to_broadcast(
                    (128, K_DIM, NG, B_PER)
                ),
                OP.mult,
            )
            for c in pool_cs:
                issue_xc(c, nc.gpsimd, uTs)
            for c in dve_cs:
                issue_xc(c, nc.vector, uTs)
                issue_s(c)
            for c in pool_cs:
                issue_s(c)

        # ---------------- main flow ----------------
        import os
        kstage = int(os.environ.get("KSTAGE", "99"))
        s_phase_s0()
        squash(final=False)
        if kstage >= 1:
            for j in range(2):
                build_vT()
                bts = []
                for i in range(5):
                    bti = smp.tile(
                        [128, NG, B_PER], bf16, tag=f"sm{i}", bufs=1,
                        name=f"bt{i}",
                    )
                    bts.append(bti)
                agr_phase(j, bts)
                if kstage == 1 + 3 * j:
                    break
                softmax_phase(bts)
                if kstage == 2 + 3 * j:
                    break
                s_phase_routed()
                squash(final=(j == 1))
                if kstage == 3 + 3 * j:
                    break
        nc.sync.dma_start(v_out[:], v_sb[:])

        for pool in (itp, smp, psA, psT, psS, perm):
            try:
                pool.release()
            except Exception:
                pass

    nc.compile()
    return nc


def _consts():
    import ml_dtypes

    bf = ml_dtypes.bfloat16
    ones32 = np.zeros((128, 32), dtype=np.float32)
    for p in range(128):
        ones32[p, p % 32] = 1.0
    return {
        "eye128": np.eye(128, dtype=np.float32),
        "ones32": ones32.astype(bf),
    }


def _prep_w(W0):
    """Host-side layout marshalling of the replicated weights (pure
    permutation + bf16 cast; done once, shared by all cores)."""
    import ml_dtypes

    bf = ml_dtypes.bfloat16
    W0 = np.ascontiguousarray(W0, dtype=np.float32)  # [1152, 10, 16, 8]
    wsk = np.ascontiguousarray(
        W0.reshape(NG, 128, C_CLS, D_DIM, K_DIM).transpose(1, 4, 0, 2, 3)
    ).astype(bf)  # [128, k, g, c, d]
    def _t2(block):  # [i, c, d, k] -> [16c+d, h, jj, (k', i'')]
        t = block.transpose(1, 2, 3, 0).reshape(128, K_DIM, I_CAPS)
        t = t.reshape(128, 2, 4, NJ, 32).transpose(0, 1, 3, 2, 4)
        return np.ascontiguousarray(t.reshape(128, 2, NJ, 128)).astype(bf)

    wt = _t2(W0[:, 0:8])    # rows 16c+d, classes 0-7
    wtb = _t2(W0[:, 2:10])  # rows 16(c-2)+d; classes 8,9 at 96:128
    return wsk, wt, wtb


def _prep_u(ush):
    import ml_dtypes

    return np.ascontiguousarray(
        ush.reshape(B_PER, NG, 128, K_DIM).transpose(2, 3, 1, 0)
    ).astype(ml_dtypes.bfloat16)  # [128, k, g, b]


def _prep_u2(ush):
    """[p=(k'4,i''32), (h2, j36, b)] with k = 4h+k', i = 32j+i''."""
    import ml_dtypes

    t = ush.reshape(B_PER, NJ, 32, K_DIM)  # [b, j, i'', k]
    t = t.transpose(3, 2, 1, 0)            # [k, i'', j, b]
    t = t.reshape(2, 4, 32, NJ, B_PER)     # [h, k', i'', j, b]
    t = t.transpose(1, 2, 0, 3, 4)         # [k', i'', h, j, b]
    return np.ascontiguousarray(t.reshape(128, 2, NJ, B_PER)).astype(
        ml_dtypes.bfloat16
    )


def get_nc():
    if "nc" not in _CACHE:
        _CACHE["nc"] = _build()
    return _CACHE["nc"]


def make_in_maps(u, W):
    consts = _consts()
    wsk, wt, wtb = _prep_w(W[0])
    in_maps = []
    for core in range(N_CORES):
        sh = np.ascontiguousarray(
            u[core * B_PER : (core + 1) * B_PER], dtype=np.float32
        )
        in_maps.append(
            {
                "uTk_h": _prep_u(sh),
                "uT2_h": _prep_u2(sh),
                "wsk_h": wsk,
                "wt_h": wt,
                "wtb_h": wtb,
                **consts,
            }
        )
    return in_maps


def kernel(u: np.ndarray, W: np.ndarray) -> np.ndarray:
    from concourse.bass_utils import run_bass_kernel_spmd

    nc = get_nc()
    in_maps = make_in_maps(u, W)
    res = run_bass_kernel_spmd(nc, in_maps, list(range(N_CORES)))
    out = np.concatenate([res.results[i]["v"] for i in range(N_CORES)], axis=0)
    return out.astype(np.float32)
